# revision 10
# baseline (speedup 1.0000x reference)
"""Trainium2 Bass kernel: multi-head attention (B=2, S=2048, D=1024, H=16, dh=64).

Returns (output, attn_prob) like the reference.

Sharding (8 cores): 2 batches x 4 head-groups. Each core handles one batch and
4 heads (as 2 head-pairs packed into 128 partitions).

Per-core plan:
  - Host supplies q/k/v transposed ([1024, 2048] per batch) plus this core's
    weight column-slices (w_q pre-scaled by 1/sqrt(dh)).
  - Projections on PE: qh_T/kh_T in [head_dim, tok] layout; vh_T transposed on
    PE into vh [tok, head_dim] (natural) layout for the context matmul.
  - Scores computed in BOTH layouts (cheap K=64 matmuls, row-tiled 2 heads
    concurrently): S=[q,k] -> exp (+row sums via ACT accumulate) -> normalize
    (DVE) -> DMA out as attn_prob; S_T=[k,q] -> exp -> context matmul
    (col-tiled 2 heads) accumulating ctx_T=[head_dim, q] in PSUM.
  - Per-head softmax normalization of ctx_T via a [128,128] transpose dance
    (normalizer is per-q, which is the free axis of ctx_T).
  - Output projection from ctx_T (already the lhsT layout); partial outputs
    summed on host across the 4 head-group cores, + b_o.
"""

import sys

for _p in ("/opt/trn_rl_repo", "/root/.axon_site/_ro/trn_rl_repo"):
    if _p not in sys.path:
        sys.path.append(_p)

from contextlib import ExitStack

import numpy as np

import concourse.bass as bass
import concourse.tile as tile
from concourse import bacc, mybir
from concourse.masks import make_identity

F32 = mybir.dt.float32
AF = mybir.ActivationFunctionType

# Problem shape (hardcoded per contract)
B, S, DH, H, DHEAD = 2, 2048, 1024, 16, 64
P = 128
KC = DH // P  # 8 hidden-dim chunks
NQ = S // P  # 16 token tiles
NPAIR = 2  # head-pairs per core (4 heads)
NCORE = 8
GCOLS = NPAIR * P  # 256 head-dim columns per core


def _mha_tile(ctx, tc, qT, kT, vT, wq, wk, wv, bq, bk, bv, wo, attn_p, out_p):
    nc = tc.nc
    CW = min(1024, S)  # free-dim chunk width for psum tiles
    NCH = S // CW  # chunks per row
    NMM = CW // 512 if CW >= 512 else 1  # 512-wide matmuls per chunk
    MMW = min(512, CW)  # matmul free width

    persist = ctx.enter_context(tc.tile_pool(name="persist", bufs=1))
    ident = persist.tile([P, P], F32, tag="ident", name="ident")
    make_identity(nc, ident)

    qhT = [persist.tile([P, S], F32, tag=f"qhT{p}", name=f"qhT{p}") for p in range(NPAIR)]
    khT = [persist.tile([P, S], F32, tag=f"khT{p}", name=f"khT{p}") for p in range(NPAIR)]
    vhT = [persist.tile([P, S], F32, tag=f"vhT{p}", name=f"vhT{p}") for p in range(NPAIR)]
    vh = persist.tile([P, NQ, NPAIR * P], F32, tag="vh", name="vh")  # [128, 16, 256]
    ctxn = [persist.tile([P, S], F32, tag=f"ctxn{p}", name=f"ctxn{p}") for p in range(NPAIR)]
    # reciprocal row-sums, indexed [q_part, (pair*2+h)*NQ + q_tile]
    recips = persist.tile([P, NPAIR * 2 * NQ], F32, tag="recips", name="recips")
    wo_sb = [persist.tile([P, DH], F32, tag=f"wo{p}", name=f"wo{p}") for p in range(NPAIR)]
    for p in range(NPAIR):
        nc.sync.dma_start(out=wo_sb[p], in_=wo[p * P : (p + 1) * P, :])

    # ---------------- Phase 1: projections ----------------
    with (
        tc.tile_pool(name="wpool", bufs=1) as wpool,
        tc.tile_pool(name="xin", bufs=3) as xin,
        tc.tile_pool(name="pj", bufs=4, space="PSUM") as pj,
    ):
        w_sb = {}
        b_sb = {}
        for nm, w, b in (("q", wq, bq), ("k", wk, bk), ("v", wv, bv)):
            for p in range(NPAIR):
                wt = wpool.tile([P, KC, P], F32, tag=f"w{nm}{p}", name=f"w{nm}{p}")
                nc.sync.dma_start(
                    out=wt,
                    in_=w[:, p * P : (p + 1) * P].rearrange(
                        "(kc part) hd -> part kc hd", part=P
                    ),
                )
                w_sb[nm, p] = wt
                bt = wpool.tile([P, 1], F32, tag=f"b{nm}{p}", name=f"b{nm}{p}")
                nc.sync.dma_start(out=bt, in_=b[p * P : (p + 1) * P, :])
                b_sb[nm, p] = bt

        for nm, x, outs in (("q", qT, qhT), ("k", kT, khT), ("v", vT, vhT)):
            psg = [
                pj.tile([P, CW], F32, tag="pj", name=f"pj_{nm}_{i}")
                for i in range(NPAIR * NCH)
            ]  # index = pair*NCH + half
            for kc in range(KC):
                xt = xin.tile([P, S], F32, tag="xt", name=f"xt_{nm}_{kc}")
                nc.sync.dma_start(out=xt, in_=x[kc * P : (kc + 1) * P, :])
                for p in range(NPAIR):
                    for half in range(NCH):
                        for n in range(NMM):
                            lo = half * CW + n * MMW
                            nc.tensor.matmul(
                                psg[p * NCH + half][:, n * MMW : (n + 1) * MMW],
                                lhsT=w_sb[nm, p][:, kc, :],
                                rhs=xt[:, lo : lo + MMW],
                                start=(kc == 0),
                                stop=(kc == KC - 1),
                            )
            for p in range(NPAIR):
                for half in range(NCH):
                    nc.scalar.activation(
                        out=outs[p][:, half * CW : (half + 1) * CW],
                        in_=psg[p * NCH + half][:, :],
                        func=AF.Identity,
                        bias=b_sb[nm, p],
                    )

    # ---------------- Phase 2: attention ----------------
    with (
        tc.tile_pool(name="sc", bufs=2, space="PSUM") as sc,
        tc.tile_pool(name="pctx", bufs=2, space="PSUM") as pctx,
        tc.tile_pool(name="ep", bufs=3) as ep,
        tc.tile_pool(name="etp", bufs=3) as etp,
        tc.tile_pool(name="cu", bufs=2) as cu,
        tc.tile_pool(name="sm", bufs=6) as sm,
        tc.tile_pool(name="op", bufs=3) as op,
    ):
        # vh (natural [tok, hd] layout) from vhT via PE transpose
        for p in range(NPAIR):
            for i in range(NQ):
                vtp = sc.tile([P, P], F32, tag="sc", name=f"vtp{p}_{i}")
                nc.tensor.transpose(vtp, vhT[p][:, i * P : (i + 1) * P], ident)
                nc.scalar.activation(
                    out=vh[:, i, p * P : (p + 1) * P], in_=vtp, func=AF.Copy
                )

        for p in range(NPAIR):
            # ctx_T accumulators, one per q-chunk: [128 (2 heads x 64 hd), CW q]
            cx = [
                pctx.tile([P, CW], F32, tag="ctx", name=f"cx{p}_{half}")
                for half in range(NCH)
            ]
            for i in range(NQ):
                # ---- S = [q, k] path (q_tile = i): attn_prob output
                for h in range(2):
                    hp = h * DHEAD  # partition base of this head
                    e = ep.tile([P, S], F32, tag="e", name=f"e{p}_{i}_{h}")
                    ssum = sm.tile([P, NCH], F32, tag="ssum", name=f"ssum{p}_{i}_{h}")
                    for c in range(NCH):
                        ps = sc.tile([P, CW], F32, tag="sc", name=f"s{p}_{i}_{h}_{c}")
                        for n in range(NMM):
                            lo = c * CW + n * MMW
                            nc.tensor.matmul(
                                ps[:, n * MMW : (n + 1) * MMW],
                                lhsT=qhT[p][hp : hp + DHEAD, i * P : (i + 1) * P],
                                rhs=khT[p][hp : hp + DHEAD, lo : lo + MMW],
                                start=True,
                                stop=True,
                            )
                        nc.scalar.activation(
                            out=e[:, c * CW : (c + 1) * CW],
                            in_=ps,
                            func=AF.Exp,
                            accum_out=ssum[:, c : c + 1],
                        )
                    ridx = (p * 2 + h) * NQ + i
                    if NCH == 1:
                        nc.vector.reciprocal(
                            out=recips[:, ridx : ridx + 1], in_=ssum[:, 0:1]
                        )
                    else:
                        rsum = sm.tile([P, 1], F32, tag="rsum", name=f"rsum{p}_{i}_{h}")
                        nc.vector.tensor_add(
                            out=rsum, in0=ssum[:, 0:1], in1=ssum[:, 1:2]
                        )
                        for c in range(2, NCH):
                            nc.vector.tensor_add(
                                out=rsum, in0=rsum, in1=ssum[:, c : c + 1]
                            )
                        nc.vector.reciprocal(out=recips[:, ridx : ridx + 1], in_=rsum)
                    nc.vector.tensor_scalar_mul(
                        out=e, in0=e, scalar1=recips[:, ridx : ridx + 1]
                    )
                    nc.sync.dma_start(
                        out=attn_p[p * 2 + h, i * P : (i + 1) * P, :], in_=e
                    )
                # ---- S_T = [k, q] path (k_tile = i) + ctx accumulation
                for h in range(2):
                    hp = h * DHEAD
                    et = etp.tile([P, S], F32, tag="et", name=f"et{p}_{i}_{h}")
                    for c in range(NCH):
                        ps2 = sc.tile([P, CW], F32, tag="sc", name=f"st{p}_{i}_{h}_{c}")
                        for n in range(NMM):
                            lo = c * CW + n * MMW
                            nc.tensor.matmul(
                                ps2[:, n * MMW : (n + 1) * MMW],
                                lhsT=khT[p][hp : hp + DHEAD, i * P : (i + 1) * P],
                                rhs=qhT[p][hp : hp + DHEAD, lo : lo + MMW],
                                start=True,
                                stop=True,
                            )
                        nc.scalar.activation(
                            out=et[:, c * CW : (c + 1) * CW], in_=ps2, func=AF.Exp
                        )
                    for half in range(NCH):
                        for n in range(NMM):
                            lo = half * CW + n * MMW
                            # skip_group_check: the sim's zero-region group
                            # check mis-addresses partition-sliced PSUM groups
                            # (execution's per-partition pending-zero model is
                            # correct; the two heads touch disjoint partitions).
                            nc.tensor.matmul(
                                cx[half][hp : hp + DHEAD, n * MMW : (n + 1) * MMW],
                                lhsT=vh[:, i, p * P + hp : p * P + hp + DHEAD],
                                rhs=et[:, lo : lo + MMW],
                                start=(i == 0),
                                stop=(i == NQ - 1),
                                skip_group_check=True,
                            )
            # ---- normalize ctx_T per head via transpose dance
            cub = cu.tile([P, S], F32, tag="cu", name=f"cu{p}")
            for half in range(NCH):
                nc.scalar.activation(
                    out=cub[:, half * CW : (half + 1) * CW],
                    in_=cx[half],
                    func=AF.Copy,
                )
            for i in range(NQ):
                tp1 = sc.tile([P, P], F32, tag="sc", name=f"tp1{p}_{i}")
                nc.tensor.transpose(tp1, cub[:, i * P : (i + 1) * P], ident)
                cq = sm.tile([P, P], F32, tag="cq", name=f"cq{p}_{i}")
                for h in range(2):
                    ridx = (p * 2 + h) * NQ + i
                    nc.vector.tensor_scalar_mul(
                        out=cq[:, h * DHEAD : (h + 1) * DHEAD],
                        in0=tp1[:, h * DHEAD : (h + 1) * DHEAD],
                        scalar1=recips[:, ridx : ridx + 1],
                    )
                tp2 = sc.tile([P, P], F32, tag="sc", name=f"tp2{p}_{i}")
                nc.tensor.transpose(tp2, cq, ident)
                nc.scalar.activation(
                    out=ctxn[p][:, i * P : (i + 1) * P], in_=tp2, func=AF.Copy
                )

        # ---- output projection (partial over this core's heads)
        for i in range(NQ):
            po = sc.tile([P, 1024], F32, tag="sc", name=f"po{i}")
            for p in range(NPAIR):
                for n in range(2):
                    nc.tensor.matmul(
                        po[:, n * 512 : (n + 1) * 512],
                        lhsT=ctxn[p][:, i * P : (i + 1) * P],
                        rhs=wo_sb[p][:, n * 512 : (n + 1) * 512],
                        start=(p == 0),
                        stop=(p == NPAIR - 1),
                    )
            ob = op.tile([P, DH], F32, tag="ob", name=f"ob{i}")
            nc.scalar.activation(out=ob, in_=po, func=AF.Copy)
            nc.sync.dma_start(out=out_p[i * P : (i + 1) * P, :], in_=ob)


def build_mha_nc():
    nc = bacc.Bacc("TRN2", target_bir_lowering=False)
    qT = nc.declare_dram_parameter("qT", [DH, S], F32, isOutput=False)
    kT = nc.declare_dram_parameter("kT", [DH, S], F32, isOutput=False)
    vT = nc.declare_dram_parameter("vT", [DH, S], F32, isOutput=False)
    wq = nc.declare_dram_parameter("wq", [DH, GCOLS], F32, isOutput=False)
    wk = nc.declare_dram_parameter("wk", [DH, GCOLS], F32, isOutput=False)
    wv = nc.declare_dram_parameter("wv", [DH, GCOLS], F32, isOutput=False)
    bq = nc.declare_dram_parameter("bq", [GCOLS, 1], F32, isOutput=False)
    bk = nc.declare_dram_parameter("bk", [GCOLS, 1], F32, isOutput=False)
    bv = nc.declare_dram_parameter("bv", [GCOLS, 1], F32, isOutput=False)
    wo = nc.declare_dram_parameter("wo", [GCOLS, DH], F32, isOutput=False)
    attn_p = nc.declare_dram_parameter("attn_p", [4, S, S], F32, isOutput=True)
    out_p = nc.declare_dram_parameter("out_p", [S, DH], F32, isOutput=True)

    with tile.TileContext(nc) as tc:
        with ExitStack() as ctx:
            _mha_tile(
                ctx, tc, qT, kT, vT, wq, wk, wv, bq, bk, bv, wo, attn_p, out_p
            )
    nc.compile()
    return nc


_NC_CACHE = None


def _get_nc():
    global _NC_CACHE
    if _NC_CACHE is None:
        _NC_CACHE = build_mha_nc()
    return _NC_CACHE


def make_in_maps(q, k, v, w_q, b_q, w_k, b_k, w_v, b_v, w_o):
    """Shard full inputs into per-core input maps."""
    f = lambda a: np.asarray(a, dtype=np.float32)
    q, k, v = f(q), f(k), f(v)
    scale = 1.0 / np.sqrt(np.float32(DHEAD))
    wqs, bqs = f(w_q) * scale, f(b_q) * scale
    wk_, bk_ = f(w_k), f(b_k)
    wv_, bv_ = f(w_v), f(b_v)
    wo_ = f(w_o)

    qT = [np.ascontiguousarray(q[b].T) for b in range(B)]
    kT = [np.ascontiguousarray(k[b].T) for b in range(B)]
    vT = [np.ascontiguousarray(v[b].T) for b in range(B)]

    in_maps = []
    for c in range(NCORE):
        b, g = divmod(c, 4)
        cols = slice(g * GCOLS, (g + 1) * GCOLS)
        in_maps.append(
            {
                "qT": qT[b],
                "kT": kT[b],
                "vT": vT[b],
                "wq": np.ascontiguousarray(wqs[:, cols]),
                "wk": np.ascontiguousarray(wk_[:, cols]),
                "wv": np.ascontiguousarray(wv_[:, cols]),
                "bq": np.ascontiguousarray(bqs[cols]).reshape(GCOLS, 1),
                "bk": np.ascontiguousarray(bk_[cols]).reshape(GCOLS, 1),
                "bv": np.ascontiguousarray(bv_[cols]).reshape(GCOLS, 1),
                "wo": np.ascontiguousarray(wo_[cols, :]),
            }
        )
    return in_maps


def gather_outputs(results, b_o):
    attn = np.empty((B, H, S, S), np.float32)
    out = np.zeros((B, S, DH), np.float32)
    for c in range(NCORE):
        b, g = divmod(c, 4)
        attn[b, g * 4 : (g + 1) * 4] = results[c]["attn_p"]
        out[b] += results[c]["out_p"]
    out += np.asarray(b_o, dtype=np.float32)
    return out, attn


def kernel(q, k, v, attn_mask, w_q, b_q, w_k, b_k, w_v, b_v, w_o, b_o):
    # attn_mask is all-False per the problem spec; masking is a no-op.
    from concourse.bass_utils import run_bass_kernel_spmd

    nc = _get_nc()
    in_maps = make_in_maps(q, k, v, w_q, b_q, w_k, b_k, w_v, b_v, w_o)
    res = run_bass_kernel_spmd(nc, in_maps, list(range(NCORE))).results
    return gather_outputs(res, b_o)
